# revision 26
# baseline (speedup 1.0000x reference)
"""Trainium2 Bass kernel for the Actor net (patch relabel + MLP), 8-core SPMD.

Strategy: data-parallel over the B*7396 patch-row axis. Host extracts the
3x3 non-overlapping patches (offset decoded from x[0,0,0,0]) into a
feature-major tensor featT [144, rows], sharded by rows across 8 cores.
The device kernel does the per-patch unique-rank relabel of channel 0
(formulated as small matmuls + elementwise ops) and the MLP, producing
[8, rows] (4 mean + 4 log_std outputs per patch row). The last linear
layer and the two heads are folded into one [256,8] matrix on the host
(no nonlinearity between them); output biases are added on the host.
"""
import sys

sys.path.insert(0, "/opt/trn_rl_repo")

import numpy as np

H = W = 256
PATCH = 3
PH = 86
C = 16
B = 32
NCORES = 8
M_TOTAL = B * PH * PH            # 236672 patch rows
NC_CORE = M_TOTAL // NCORES      # 29584 rows per core
TILE_N = 512
TPS = 4                          # 512-tiles per super-tile
SUP = TILE_N * TPS               # 2048
NSUP = 15
NCP = NSUP * SUP                 # 30720 padded columns per core
NT = NCP // TILE_N               # 60
TPK = 3                          # tiles packed per 128-partition group
NCP3 = NCP // TPK                # 10240 (packed-by-3 column space)
D_IN = PATCH * PATCH * C         # 144

TRACE = False
TRACE_KWARGS = {}
LAST_EXEC_NS = None
LAST_RESULT = None

# feature permutation: 9 ch0 features first (patch positions), then the rest
_PERM = [p * C for p in range(9)] + [p * C + c for p in range(9) for c in range(1, C)]


def _relabel_consts():
    EAB = np.zeros((9, 81), np.float32)   # psAB[j*9+i] = a_i - a_j
    EA = np.zeros((9, 81), np.float32)    # FD[j*9+i] = fd_i
    TLT = np.zeros((81, 9), np.float32)   # S_j = sum_{i<j} eq[j*9+i]
    TALL = np.zeros((81, 9), np.float32)  # uni_j = sum_i prod[j*9+i]
    TINC = np.zeros((9, 9), np.float32)   # d_m = sum_{k<=m} fo_k
    for j in range(9):
        for i in range(9):
            q = j * 9 + i
            EAB[i, q] += 1.0
            EAB[j, q] -= 1.0
            EA[i, q] = 1.0
            TALL[q, j] = 1.0
            if i < j:
                TLT[q, j] = 1.0
    for k in range(9):
        for m in range(9):
            if k <= m:
                TINC[k, m] = 1.0
    return EAB, EA, TLT, TALL, TINC


_GRAPH = None


def _build_graph():
    global _GRAPH
    if _GRAPH is not None:
        return _GRAPH
    import concourse.bass as bass
    import concourse.bacc as bacc
    import concourse.mybir as mybir
    import concourse.tile as tile

    bf16 = mybir.dt.bfloat16
    f32 = mybir.dt.float32
    AF = mybir.ActivationFunctionType
    ALU = mybir.AluOpType

    nc = bacc.Bacc("TRN2", use_seq_codegen=True)
    feat_e = nc.declare_dram_parameter("feat", [D_IN, NCP], bf16, isOutput=False)
    w1_e = nc.declare_dram_parameter("w1", [144, 256], bf16, isOutput=False)
    w2_e = nc.declare_dram_parameter("w2", [256, 256], bf16, isOutput=False)
    whx_e = nc.declare_dram_parameter("whx", [256, 8], bf16, isOutput=False)
    b1_e = nc.declare_dram_parameter("b1", [256, 1], f32, isOutput=False)
    b2_e = nc.declare_dram_parameter("b2", [256, 1], f32, isOutput=False)
    eab_e = nc.declare_dram_parameter("eab", [9, 81], bf16, isOutput=False)
    ea_e = nc.declare_dram_parameter("ea", [9, 81], bf16, isOutput=False)
    tlt_e = nc.declare_dram_parameter("tlt", [81, 9], bf16, isOutput=False)
    tall_e = nc.declare_dram_parameter("tall", [81, 9], bf16, isOutput=False)
    tinc_e = nc.declare_dram_parameter("tinc", [9, 9], bf16, isOutput=False)
    bfo_e = nc.declare_dram_parameter("bfo", [128, 1], f32, isOutput=False)
    out_e = nc.declare_dram_parameter("out", [8, NCP], f32, isOutput=True)

    def ts(i, n=TILE_N):
        return bass.ts(i, n)

    with tile.TileContext(nc) as tc:
        with (
            tc.tile_pool(name="const", bufs=1) as cp,
            tc.tile_pool(name="sb", bufs=3) as sb,
        ):
            def const_tile(src, shape, dtype, tag, eng=None):
                t = cp.tile(shape, dtype, tag=tag, name=tag)
                (eng or nc.sync).dma_start(t[:], src)
                return t

            gq = nc.gpsimd
            w1_00 = const_tile(w1_e[0:128, 0:128], [128, 128], bf16, "w1_00", gq)
            w1_01 = const_tile(w1_e[0:128, 128:256], [128, 128], bf16, "w1_01", gq)
            w1_10 = cp.tile([128, 128], bf16, tag="w1_10", name="w1_10")
            w1_11 = cp.tile([128, 128], bf16, tag="w1_11", name="w1_11")
            for t2 in range(2):
                gq.dma_start(w1_10[64 * t2:64 * t2 + 16, :], w1_e[128:144, 0:128])
                gq.dma_start(w1_11[64 * t2:64 * t2 + 16, :], w1_e[128:144, 128:256])
            w2_00 = const_tile(w2_e[0:128, 0:128], [128, 128], bf16, "w2_00", gq)
            w2_01 = const_tile(w2_e[0:128, 128:256], [128, 128], bf16, "w2_01", gq)
            w2_10 = const_tile(w2_e[128:256, 0:128], [128, 128], bf16, "w2_10", gq)
            w2_11 = const_tile(w2_e[128:256, 128:256], [128, 128], bf16, "w2_11", gq)
            whx0 = const_tile(whx_e[0:128, :], [128, 8], bf16, "whx0", gq)
            whx1 = const_tile(whx_e[128:256, :], [128, 8], bf16, "whx1", gq)
            b1a = const_tile(b1_e[0:128, :], [128, 1], f32, "b1a", gq)
            b1b = const_tile(b1_e[128:256, :], [128, 1], f32, "b1b", gq)
            b2a = const_tile(b2_e[0:128, :], [128, 1], f32, "b2a", gq)
            b2b = const_tile(b2_e[128:256, :], [128, 1], f32, "b2b", gq)
            # AB weights first on sync (they gate the first relabel matmul);
            # the rest of the small consts go via the scalar-engine queue.
            eab = cp.tile([128, 81], bf16, tag="eab", name="eab")
            ea = cp.tile([128, 81], bf16, tag="ea", name="ea")
            tinc = cp.tile([128, 9], bf16, tag="tinc", name="tinc")
            for t in range(TPK):
                nc.sync.dma_start(eab[32 * t:32 * t + 9, :], eab_e[:])
            tlt = const_tile(tlt_e[:], [81, 9], bf16, "tlt", nc.scalar)
            tall = const_tile(tall_e[:], [81, 9], bf16, "tall", nc.scalar)
            bfo = const_tile(bfo_e[:], [128, 1], f32, "bfo", nc.scalar)
            for t in range(TPK):
                nc.scalar.dma_start(ea[32 * t:32 * t + 9, :], ea_e[:])
                nc.scalar.dma_start(tinc[32 * t:32 * t + 9, :], tinc_e[:])

            # channel-0 buffers packed 3 tiles per partition group: partition
            # 32*t+p holds position p of 512-tile (3g+t), columns g*512+n.
            NG = NT // TPK
            ch0B = sb.tile([128, NCP3], bf16, tag="ch0B", bufs=1, name="ch0B")
            uniB = sb.tile([128, NCP3], bf16, tag="uniB", bufs=1, name="uniB")
            GCH = 2   # groups per startup-load chunk
            for c in range(NG // GCH):
                for t in range(TPK):
                    nc.sync.dma_start(
                        ch0B[32 * t:32 * t + 9, c * GCH * TILE_N:
                             (c + 1) * GCH * TILE_N].rearrange(
                                 "p (g n) -> p g n", g=GCH),
                        feat_e[0:9, :].rearrange(
                            "p (g q n) -> p g (q n)", g=NG,
                            q=TPK)[:, c * GCH:(c + 1) * GCH,
                                   t * TILE_N:(t + 1) * TILE_N])

            def in_rng(j):
                return 0 <= j < NT

            # ---- phase 1: unique-rank relabel, software-pipelined ----
            psAB = {}; eqt = {}; psS3 = {}; fo3 = {}; psD3 = {}
            fd3 = {}; psFD = {}; psU3 = {}

            ps_ctx = tc.tile_pool(name="ps_rl", bufs=2, space=bass.MemorySpace.PSUM)
            ps = ps_ctx.__enter__()
            for k in range(NT + 19):
                j = k
                if in_rng(j):  # AB: pairwise differences a_i - a_j
                    t, g = j % TPK, j // TPK
                    psAB[j] = ps.tile([81, TILE_N], f32, tag="psAB", bufs=2,
                                      name=f"psAB_{j}")
                    nc.tensor.matmul(psAB[j][:], eab[32 * t:32 * t + 9, :],
                                     ch0B[32 * t:32 * t + 9, ts(g)],
                                     start=True, stop=True,
                                     tile_position=(32 * t, 0))
                j = k - 1
                if in_rng(j):  # ABS (ACT): |a_i - a_j|
                    eqt[j] = sb.tile([81, TILE_N], bf16, tag="eq", bufs=18,
                                     name=f"eqt_{j}")
                    nc.scalar.activation(eqt[j][:], psAB[j][:], AF.Abs)
                    del psAB[j]
                j = k - 2
                if in_rng(j):  # MIN (DVE): m = min(|d|, 1)
                    nc.vector.tensor_scalar(eqt[j][:], eqt[j][:], 1.0, None,
                                            op0=ALU.min)
                j = k - 3
                if in_rng(j) and j % TPK == TPK - 1:  # S x3 then FO (packed)
                    g = j // TPK
                    psS3[g] = ps.tile([128, TILE_N], f32, tag="S3", bufs=2,
                                      name=f"psS3_{g}")
                    for t in range(TPK):
                        nc.tensor.matmul(psS3[g][32 * t:32 * t + 9, :], tlt[:],
                                         eqt[TPK * g + t][:], start=True, stop=True,
                                         tile_position=(0, 32 * t))
                    fo3[g] = sb.tile([128, TILE_N], bf16, tag="fo3", bufs=2,
                                     name=f"fo3_{g}")
                    nc.scalar.activation(fo3[g][:], psS3[g][:], AF.Relu,
                                         bias=bfo[:])
                    del psS3[g]
                j = k - 6
                if in_rng(j) and j % TPK == TPK - 1:  # D x3 then FD2 (packed)
                    g = j // TPK
                    psD3[g] = ps.tile([128, TILE_N], f32, tag="D3", bufs=1,
                                      name=f"psD3_{g}")
                    for t in range(TPK):
                        nc.tensor.matmul(psD3[g][32 * t:32 * t + 9, :],
                                         tinc[32 * t:32 * t + 9, :],
                                         fo3[g][32 * t:32 * t + 9, :],
                                         start=True, stop=True,
                                         tile_position=(32 * t, 32 * t))
                    fd3[g] = sb.tile([128, TILE_N], bf16, tag="fd3", bufs=2,
                                     name=f"fd3_{g}")
                    nc.vector.scalar_tensor_tensor(
                        fd3[g][:], fo3[g][:], 0.0, psD3[g][:],
                        op0=ALU.bypass, op1=ALU.mult)
                    del psD3[g]; del fo3[g]
                j = k - 10
                if in_rng(j):  # FD expand
                    t, g = j % TPK, j // TPK
                    psFD[j] = ps.tile([81, TILE_N], f32, tag="psFD", bufs=2,
                                      name=f"psFD_{j}")
                    nc.tensor.matmul(psFD[j][:], ea[32 * t:32 * t + 9, :],
                                     fd3[g][32 * t:32 * t + 9, :],
                                     start=True, stop=True,
                                     tile_position=(32 * t, 0))
                    if t == TPK - 1:
                        del fd3[g]
                j = k - 11
                if in_rng(j):  # PR: prod = (m-1)*FD, in place over m
                    nc.vector.scalar_tensor_tensor(
                        eqt[j][:], eqt[j][:], 1.0, psFD[j][:],
                        op0=ALU.subtract, op1=ALU.mult)
                    del psFD[j]
                j = k - 15
                if in_rng(j) and j % TPK == TPK - 1:  # U x3 then CH (packed)
                    g = j // TPK
                    psU3[g] = ps.tile([128, TILE_N], f32, tag="U3", bufs=1,
                                      name=f"psU3_{g}")
                    for t in range(TPK):
                        nc.tensor.matmul(psU3[g][32 * t:32 * t + 9, :], tall[:],
                                         eqt[TPK * g + t][:], start=True, stop=True,
                                         tile_position=(0, 32 * t))
                        del eqt[TPK * g + t]
                    # uni masked by (raw ch0 != 0), packed write
                    nc.vector.scalar_tensor_tensor(
                        uniB[:, ts(g)], ch0B[:, ts(g)], 0.0, psU3[g][:],
                        op0=ALU.not_equal, op1=ALU.mult)
                    del psU3[g]
            ps_ctx.__exit__(None, None, None)

            # ---- phase 2: MLP, software-pipelined ----
            fa = {}; fb = {}; ps1a = {}; ps1b = {}; h1a = {}; h1b = {}
            ps2a = {}; ps2b = {}; h2a = {}; h2b = {}; psH = {}; otS = {}

            ps_ctx = tc.tile_pool(name="ps_mlp", bufs=2, space=bass.MemorySpace.PSUM)
            ps = ps_ctx.__enter__()
            for k in range(NT + 17):
                if k % TPS == 0 and in_rng(k):  # LD super
                    s = k // TPS
                    fa[s] = sb.tile([128, SUP], bf16, tag="fa", bufs=3,
                                    name=f"fa_{s}")
                    fb[s] = sb.tile([128, SUP // 2], bf16, tag="fb", bufs=3,
                                    name=f"fb_{s}")
                    nc.sync.dma_start(fa[s][:], feat_e[0:128, ts(s, SUP)])
                    # fb packed: partition block 64*t2 holds tile (2u+t2)
                    fbsrc = feat_e[128:144, ts(s, SUP)].rearrange(
                        "p (u t n) -> p u t n", u=2, t=2)
                    for t2 in range(2):
                        nc.sync.dma_start(
                            fb[s][64 * t2:64 * t2 + 16, :].rearrange(
                                "p (u n) -> p u n", u=2),
                            fbsrc[:, :, t2, :])
                j = k - 5
                if in_rng(j):  # OV: overlay relabeled ch0 (plain copy)
                    t3, g = j % TPK, j // TPK
                    t, s = j % TPS, j // TPS
                    nc.vector.tensor_copy(fa[s][0:9, ts(t)],
                                          uniB[32 * t3:32 * t3 + 9, ts(g)])
                j = k - 8
                if in_rng(j) and j % 2 == 1:  # L1 + R1, weight-grouped pairs
                    jj = (j - 1, j)
                    fv = {}
                    for i in jj:
                        t, s = i % TPS, i // TPS
                        t2, ul = i % 2, (i % TPS) // 2
                        fv[i] = (fa[s][:, ts(t)],
                                 fb[s][64 * t2:64 * t2 + 16, ts(ul)])
                        ps1a[i] = ps.tile([128, TILE_N], f32, tag="ps1", bufs=4,
                                          name=f"ps1a_{i}")
                        ps1b[i] = ps.tile([128, TILE_N], f32, tag="ps1", bufs=4,
                                          name=f"ps1b_{i}")
                    for i in jj:
                        nc.tensor.matmul(ps1a[i][:], w1_00[:], fv[i][0],
                                         start=True, stop=False)
                    for i in jj:
                        nc.tensor.matmul(ps1b[i][:], w1_01[:], fv[i][0],
                                         start=True, stop=False)
                    for i in jj:
                        t2 = i % 2
                        nc.tensor.matmul(ps1a[i][:],
                                         w1_10[64 * t2:64 * t2 + 16, :],
                                         fv[i][1], start=False, stop=True,
                                         tile_position=(64 * t2, 0))
                    for i in jj:
                        t2 = i % 2
                        nc.tensor.matmul(ps1b[i][:],
                                         w1_11[64 * t2:64 * t2 + 16, :],
                                         fv[i][1], start=False, stop=True,
                                         tile_position=(64 * t2, 0))
                    for i in jj:
                        h1a[i] = sb.tile([128, TILE_N], bf16, tag="h1a", bufs=4,
                                         name=f"h1a_{i}")
                        h1b[i] = sb.tile([128, TILE_N], bf16, tag="h1b", bufs=4,
                                         name=f"h1b_{i}")
                        nc.scalar.activation(h1a[i][:], ps1a[i][:], AF.Relu,
                                             bias=b1a[:])
                        nc.scalar.activation(h1b[i][:], ps1b[i][:], AF.Relu,
                                             bias=b1b[:])
                        del ps1a[i]; del ps1b[i]
                j = k - 10
                if in_rng(j):  # L2 + R2 (one relu on DVE for engine balance)
                    ps2a[j] = ps.tile([128, TILE_N], f32, tag="ps2", bufs=2,
                                      name=f"ps2a_{j}")
                    nc.tensor.matmul(ps2a[j][:], w2_00[:], h1a[j][:],
                                     start=True, stop=False)
                    nc.tensor.matmul(ps2a[j][:], w2_10[:], h1b[j][:],
                                     start=False, stop=True)
                    ps2b[j] = ps.tile([128, TILE_N], f32, tag="ps2", bufs=2,
                                      name=f"ps2b_{j}")
                    nc.tensor.matmul(ps2b[j][:], w2_01[:], h1a[j][:],
                                     start=True, stop=False)
                    nc.tensor.matmul(ps2b[j][:], w2_11[:], h1b[j][:],
                                     start=False, stop=True)
                    h2a[j] = sb.tile([128, TILE_N], bf16, tag="h2a", bufs=4,
                                     name=f"h2a_{j}")
                    h2b[j] = sb.tile([128, TILE_N], bf16, tag="h2b", bufs=4,
                                     name=f"h2b_{j}")
                    nc.scalar.activation(h2a[j][:], ps2a[j][:], AF.Relu, bias=b2a[:])
                    nc.vector.tensor_scalar(h2b[j][:], ps2b[j][:], b2b[:], 0.0,
                                            op0=ALU.add, op1=ALU.max)
                    del ps2a[j]; del ps2b[j]; del h1a[j]; del h1b[j]
                j = k - 12
                if in_rng(j):  # fused L3+heads (+ output copy)
                    t, s = j % TPS, j // TPS
                    psH[j] = ps.tile([8, TILE_N], f32, tag="psH", bufs=2,
                                     name=f"psH_{j}")
                    nc.tensor.matmul(psH[j][:], whx0[:], h2a[j][:],
                                     start=True, stop=False)
                    nc.tensor.matmul(psH[j][:], whx1[:], h2b[j][:],
                                     start=False, stop=True)
                    if t == 0:
                        otS[s] = sb.tile([8, SUP], f32, tag="otS", bufs=2,
                                         name=f"otS_{s}")
                    nc.vector.tensor_copy(otS[s][:, ts(t)], psH[j][:])
                    del psH[j]; del h2a[j]; del h2b[j]
                    if t == TPS - 1:  # ST super
                        nc.sync.dma_start(out_e[:, ts(s, SUP)], otS[s][:])
                        del otS[s]
            ps_ctx.__exit__(None, None, None)

    nc.finalize()
    _GRAPH = nc
    return nc


def _extract_features(x):
    """numpy port of the reference's offset decode + patch extraction."""
    x = np.array(x, dtype=np.float32, copy=True)
    code = x[0, 0, 0, 0]
    it = np.float32(np.mod(code, np.float32(100.0)))
    x[0, 0, 0, 0] = np.float32((code - it) / np.float32(100.0))
    it_i = np.int32(it)
    off_h = int(it_i % 3)
    off_w = int((it_i // 3) % 3)
    xp = np.zeros((B, H + 4, W + 4, C), np.float32)
    xp[:, 2:2 + H, 2:2 + W, :] = x
    xp = xp[:, 2 - off_h:2 - off_h + H + 2, 2 - off_w:2 - off_w + W + 2, :]
    patches = xp.reshape(B, PH, PATCH, PH, PATCH, C)
    patches = patches.transpose(0, 1, 3, 2, 4, 5).reshape(M_TOTAL, PATCH * PATCH, C)
    return patches.reshape(M_TOTAL, D_IN)


_BFO = np.zeros((128, 1), np.float32)
for _t in range(TPK):
    _BFO[32 * _t:32 * _t + 9, 0] = 1.0 - np.arange(9, dtype=np.float32)


def kernel(x, W1, b1, W2, b2, W3, b3, Wm, bm, Wl, bl):
    global LAST_EXEC_NS, LAST_RESULT
    from concourse.bass_utils import run_bass_kernel_spmd
    import concourse.mybir as mybir

    bf16 = mybir.dt.np(mybir.dt.bfloat16)
    feat = _extract_features(np.asarray(x))
    featP = feat[:, _PERM]

    EAB, EA, TLT, TALL, TINC = _relabel_consts()
    W1 = np.asarray(W1, np.float32)[_PERM, :]
    W3 = np.asarray(W3, np.float32)
    b3 = np.asarray(b3, np.float32)
    Wm = np.asarray(Wm, np.float32)
    Wl = np.asarray(Wl, np.float32)
    whx = W3 @ np.concatenate([Wm, Wl], axis=1)          # [256, 8]
    bias8 = np.concatenate([b3 @ Wm + np.asarray(bm, np.float32),
                            b3 @ Wl + np.asarray(bl, np.float32)])  # [8]
    common = dict(
        w1=W1.astype(bf16),
        w2=np.asarray(W2, np.float32).astype(bf16),
        whx=whx.astype(bf16),
        b1=np.asarray(b1, np.float32).reshape(256, 1),
        b2=np.asarray(b2, np.float32).reshape(256, 1),
        eab=EAB.astype(bf16), ea=EA.astype(bf16), tlt=TLT.astype(bf16),
        tall=(-TALL).astype(bf16), tinc=TINC.astype(bf16),
        bfo=_BFO,
    )
    in_maps = []
    for c in range(NCORES):
        shard = featP[c * NC_CORE:(c + 1) * NC_CORE, :]  # [NC_CORE, 144]
        ft = np.zeros((D_IN, NCP), bf16)
        ft[:, :NC_CORE] = shard.T.astype(bf16)
        in_maps.append(dict(feat=np.ascontiguousarray(ft), **common))

    nc = _build_graph()
    res = run_bass_kernel_spmd(
        nc, in_maps, list(range(NCORES)), trace=TRACE, trace_kwargs=TRACE_KWARGS)
    LAST_EXEC_NS = res.exec_time_ns
    LAST_RESULT = res
    means, logs = [], []
    for c in range(NCORES):
        o = res.results[c]["out"][:, :NC_CORE] + bias8[:, None]
        means.append(o[0:4].T.reshape(B // NCORES, PH * PH * 4))
        logs.append(o[4:8].T.reshape(B // NCORES, PH * PH * 4))
    mean = np.concatenate(means, axis=0)
    log_std = np.concatenate(logs, axis=0)
    return mean, log_std
